# revision 10
# baseline (speedup 1.0000x reference)
"""2-layer GAT on 8 Trainium2 NeuronCores (Bass/Tile) — v2.

Dst-partitioned edge-parallel in a permuted node space. The 391 dst-node
blocks (128 nodes each) are snake-packed by edge count onto 8 cores x 49
slots, giving every core identical tile counts (perfect SPMD balance, minimal
ceil-padding). Node tables are split into region A (slots < SLA on every
core) and region B so int16 gather indices fit and so the phase-A/A-half
AllGather unlock L1/L2 gathers early.

Per 128-dst-node block: one dma_gather of per-edge source rows (256 feats +
8 src scores) from the replicated node table, one dma_gather of per-edge dst
scores from the score-column window of the same table (elem_step=384,
elem_size=128), exp(leakyrelu(alpha)) written back into the gathered rows'
score columns, and ONE PSUM-accumulated matmul per 128-edge tile computes
both the weighted scatter-add numerator and the softmax denominator (the
one-hot dst matrix is stationary; per-edge weights ride the streamed rhs).
"""
import os
import numpy as np

P = 128
NCORES = 8

_CACHE = {}


def _wrap_idx_segments(segs, total_cols):
    arr = np.zeros((16, total_cols), np.int16)
    for off, idx in segs:
        n = len(idx)
        if n:
            arr[:, off:off + n // 16] = idx.reshape(n // 16, 16).T
    return np.tile(arr, (8, 1))


def _prep(edge_index, N):
    src0 = np.concatenate([np.asarray(edge_index[0], np.int64),
                           np.arange(N, dtype=np.int64)])
    dst0 = np.concatenate([np.asarray(edge_index[1], np.int64),
                           np.arange(N, dtype=np.int64)])
    NBT = (N + P - 1) // P             # real dst blocks
    NB = (NBT + NCORES - 1) // NCORES  # slots per core
    SLA = (NB + 1) // 2                # region-A slots per core
    SLB = NB - SLA
    RA, RB = NCORES * SLA * P, NCORES * SLB * P

    bc = np.bincount(dst0 // P, minlength=NBT)
    order = np.argsort(-bc, kind="stable")
    block_of = -np.ones((NCORES, NB), np.int64)
    for r in range(NB):
        chunk = order[r * NCORES:(r + 1) * NCORES]
        cores = list(range(NCORES)) if r % 2 == 0 else list(range(NCORES - 1, -1, -1))
        for i in range(len(chunk)):
            block_of[cores[i], r] = chunk[i]
    core_of = np.zeros(NBT, np.int64)
    slot_of = np.zeros(NBT, np.int64)
    for c in range(NCORES):
        for k in range(NB):
            B = block_of[c, k]
            if B >= 0:
                core_of[B] = c
                slot_of[B] = k

    # node -> (region, global table row)
    nid = np.arange(N, dtype=np.int64)
    nB, npos = nid // P, nid % P
    ncr, nk = core_of[nB], slot_of[nB]
    n_tb = (nk >= SLA).astype(np.int64)
    n_row = np.where(n_tb == 0,
                     ncr * SLA * P + nk * P + npos,
                     ncr * SLB * P + (nk - SLA) * P + npos)

    e_c = core_of[dst0 // P]
    e_k = slot_of[dst0 // P]
    e_s = n_tb[src0]
    key = (e_c * NB + e_k) * 2 + e_s
    eo = np.argsort(key, kind="stable")
    key_s = key[eo]
    srcrow_s = n_row[src0][eo]
    dpos_s = (dst0 % P)[eo]
    seg_lo = np.searchsorted(key_s, np.arange(NCORES * NB * 2))
    seg_hi = np.searchsorted(key_s, np.arange(NCORES * NB * 2) + 1)

    cnt = (seg_hi - seg_lo).reshape(NCORES, NB, 2)
    NT = np.zeros((NB, 2), np.int64)
    for k in range(NB):
        for s in range(2):
            NT[k, s] = (int(cnt[:, k, s].max()) + P - 1) // P

    tile_of = np.zeros((NB, 2), np.int64)
    t = 0
    for k in range(NB):
        for s in range(2):
            tile_of[k, s] = t
            t += int(NT[k, s])
    NTOT = max(t, 1)
    NTMAX = int(max(2, NT.sum(axis=1).max()))

    groups = [list(range(g, min(g + 2, SLA))) for g in range(0, SLA, 2)]
    groups += [list(range(g, min(g + 2, NB))) for g in range(SLA, NB, 2)]

    g_cols, g_off = 0, []
    for g, slots in enumerate(groups):
        offs = []
        for s in range(2):
            ntg = int(sum(NT[k, s] for k in slots))
            offs.append((g_cols, ntg))
            g_cols += ntg * 8
        g_off.append(offs)
    d_cols, d_off = 0, []
    for g, slots in enumerate(groups):
        ntg = int(sum(NT[k, 0] + NT[k, 1] for k in slots))
        d_off.append((d_cols, ntg))
        d_cols += ntg * 8
    g_cols = max(g_cols, 16)
    d_cols = max(d_cols, 16)

    plan = dict(N=N, NB=NB, SLA=SLA, SLB=SLB, RA=RA, RB=RB, NT=NT,
                tile_of=tile_of, NTOT=NTOT, NTMAX=NTMAX, groups=groups,
                g_off=g_off, d_off=d_off, g_cols=g_cols, d_cols=d_cols)

    per_core = []
    for c in range(NCORES):
        gsegs, dsegs = [], []
        d_fp = np.full((P, NTOT), -1.0, np.float32)
        for g, slots in enumerate(groups):
            for s in range(2):
                col0, ntg = g_off[g][s]
                if ntg == 0:
                    continue
                idx = np.zeros(ntg * P, np.int64)
                pos = 0
                for k in slots:
                    lo = seg_lo[(c * NB + k) * 2 + s]
                    hi = seg_hi[(c * NB + k) * 2 + s]
                    idx[pos:pos + hi - lo] = srcrow_s[lo:hi]
                    pos += int(NT[k, s]) * P
                gsegs.append((col0, idx.astype(np.int16)))
            col0d, dntg = d_off[g]
            didx = np.zeros(dntg * P, np.int64)
            pos = 0
            for k in slots:
                if k < SLA:
                    rbase = (c * SLA + k) * P
                else:
                    rbase = (c * SLB + (k - SLA)) * P
                for s in range(2):
                    lo = seg_lo[(c * NB + k) * 2 + s]
                    hi = seg_hi[(c * NB + k) * 2 + s]
                    nslots = int(NT[k, s]) * P
                    didx[pos:pos + hi - lo] = rbase + dpos_s[lo:hi]
                    t0 = int(tile_of[k, s])
                    dv = np.full(nslots, -1.0, np.float32)
                    dv[:hi - lo] = dpos_s[lo:hi].astype(np.float32)
                    d_fp[:, t0:t0 + int(NT[k, s])] = \
                        dv.reshape(int(NT[k, s]), P).T
                    pos += nslots
            dsegs.append((col0d, didx.astype(np.int16)))
        per_core.append(dict(
            g_idx=_wrap_idx_segments(gsegs, g_cols),
            dl_idx=_wrap_idx_segments(dsegs, d_cols),
            d_fp=d_fp,
        ))

    idsA = np.zeros(RA, np.int64)
    maskA = np.zeros(RA, bool)
    idsA[n_row[n_tb == 0]] = nid[n_tb == 0]
    maskA[n_row[n_tb == 0]] = True
    idsB = np.zeros(max(RB, 1), np.int64)
    maskB = np.zeros(max(RB, 1), bool)
    if RB:
        idsB[n_row[n_tb == 1]] = nid[n_tb == 1]
        maskB[n_row[n_tb == 1]] = True
    plan["idsA"], plan["maskA"] = idsA, maskA
    plan["idsB"], plan["maskB"] = idsB, maskB
    plan["block_of"] = block_of
    return plan, per_core


def _build(plan, dims, has_b1, has_b2):
    import concourse.bass as bass
    import concourse.bacc as bacc
    import concourse.tile as tile
    from concourse import mybir

    f32 = mybir.dt.float32
    bf16 = mybir.dt.bfloat16
    i16 = mybir.dt.int16
    AF = mybir.ActivationFunctionType
    OP = mybir.AluOpType

    NB, SLA, SLB = plan["NB"], plan["SLA"], plan["SLB"]
    RA, RB = plan["RA"], plan["RB"]
    NT, tile_of = plan["NT"], plan["tile_of"]
    NTOT, NTMAX = plan["NTOT"], plan["NTMAX"]
    groups, g_off, d_off = plan["groups"], plan["g_off"], plan["d_off"]
    HID, H1, C1, OUT = dims["HID"], dims["H1"], dims["C1"], dims["OUT"]
    W1C = HID + 2 * H1            # 272: feats | s_src | s_dst
    SROW1 = 384
    ROW2 = 128
    W2C = OUT + 2                 # 66: feats | s_src2 | s_dst2
    STR1 = HID + H1               # 264: matmul stream width, den at 256:264
    NEG = 0.2
    NPC = NB * P
    RBp = max(RB, P)

    nc = bacc.Bacc(num_devices=NCORES, num_swdge_queues=4)

    xTA = nc.dram_tensor("xTA", [2, P, RA], bf16, kind="ExternalInput")
    xTB = (nc.dram_tensor("xTB", [2, P, RB], bf16, kind="ExternalInput")
           if RB else None)
    w1e = nc.dram_tensor("w1e", [2, P, W1C], bf16, kind="ExternalInput")
    w2e = nc.dram_tensor("w2e", [2, P, W2C], bf16, kind="ExternalInput")
    negcs = nc.dram_tensor("negcs", [P, W2C], f32, kind="ExternalInput")
    g_idx_d = nc.dram_tensor("g_idx", [P, plan["g_cols"]], i16,
                             kind="ExternalInput")
    dl_idx_d = nc.dram_tensor("dl_idx", [P, plan["d_cols"]], i16,
                              kind="ExternalInput")
    d_fp_d = nc.dram_tensor("d_fp", [P, NTOT], bf16, kind="ExternalInput")
    if has_b1:
        b1_d = nc.dram_tensor("b1r", [P, HID], bf16, kind="ExternalInput")
    if has_b2:
        b2_d = nc.dram_tensor("b2r", [P, OUT], f32, kind="ExternalInput")
    out2 = nc.dram_tensor("out2", [NPC, OUT], f32, kind="ExternalOutput")

    hextA = nc.dram_tensor("hextA", [RA, SROW1], bf16)
    hextB = nc.dram_tensor("hextB", [RBp, SROW1], bf16)
    h2locA = nc.dram_tensor("h2locA", [SLA * P, ROW2], bf16)
    h2locB = nc.dram_tensor("h2locB", [max(SLB * P, P), ROW2], bf16)
    h2A = nc.dram_tensor("h2A", [RA, ROW2], bf16, addr_space="Shared")
    h2B = nc.dram_tensor("h2B", [RBp, ROW2], bf16, addr_space="Shared")

    def sub_ap(t, elem_off, dims_):
        a = t[:]
        return bass.AP(tensor=a.tensor, offset=a.offset + elem_off,
                       ap=[a.ap[0]] + dims_)

    with tile.TileContext(nc, num_cores=NCORES) as tc:
        with tc.tile_pool(name="consts", bufs=1) as cp:
            w1t, w2t = [], []
            for kh in range(2):
                t1 = cp.tile([P, W1C], bf16, tag=f"w1t{kh}")
                nc.sync.dma_start(out=t1[:], in_=w1e[kh])
                w1t.append(t1)
                t2 = cp.tile([P, W2C], bf16, tag=f"w2t{kh}")
                nc.sync.dma_start(out=t2[:], in_=w2e[kh])
                w2t.append(t2)
            ncs_t = cp.tile([P, W2C], f32)
            nc.sync.dma_start(out=ncs_t[:], in_=negcs[:])
            gidx_t = cp.tile([P, plan["g_cols"]], i16)
            nc.sync.dma_start(out=gidx_t[:], in_=g_idx_d[:])
            dlidx_t = cp.tile([P, plan["d_cols"]], i16)
            nc.sync.dma_start(out=dlidx_t[:], in_=dl_idx_d[:])
            dfp_t = cp.tile([P, NTOT], bf16)
            nc.sync.dma_start(out=dfp_t[:], in_=d_fp_d[:])
            ib_i = cp.tile([P, NTMAX * P], mybir.dt.int32)
            nc.gpsimd.iota(ib_i[:], pattern=[[0, NTMAX], [1, P]], base=0,
                           channel_multiplier=0)
            iota_big = cp.tile([P, NTMAX * P], bf16)
            nc.vector.tensor_copy(out=iota_big[:], in_=ib_i[:])
            iota_sq = cp.tile([P, P], mybir.dt.int32)
            nc.gpsimd.iota(iota_sq[:], pattern=[[1, P]], base=0,
                           channel_multiplier=0)
            iota_t = cp.tile([P, P], bf16)
            nc.vector.tensor_copy(out=iota_t[:], in_=iota_sq[:])
            pidx_i = cp.tile([P, 1], mybir.dt.int32)
            nc.gpsimd.iota(pidx_i[:], pattern=[[0, 1]], base=0,
                           channel_multiplier=1)
            pidx_f = cp.tile([P, 1], f32)
            nc.vector.tensor_copy(out=pidx_f[:], in_=pidx_i[:])
            ident = cp.tile([P, P], bf16)
            nc.vector.tensor_scalar(out=ident[:], in0=iota_t[:],
                                    scalar1=pidx_f[:], scalar2=None,
                                    op0=OP.is_equal)
            b1_t = b2_t = None
            if has_b1:
                b1_t = cp.tile([P, HID], bf16)
                nc.sync.dma_start(out=b1_t[:], in_=b1_d[:])
            if has_b2:
                b2_t = cp.tile([P, OUT], f32)
                nc.sync.dma_start(out=b2_t[:], in_=b2_d[:])

            # ---------------- phase A: node tables ----------------
            CH = 8
            with (
                tc.tile_pool(name="xc", bufs=4) as xc,
                tc.tile_pool(name="psA", bufs=2, space="PSUM") as psAp,
                tc.tile_pool(name="rowp", bufs=6) as rowp,
            ):
                def sweep(xT, hext, rows):
                    nblk = rows // P
                    for ch in range(0, nblk, CH):
                        ntc = min(CH, nblk - ch)
                        ck = []
                        for kh in range(2):
                            t_ = xc.tile([P, CH * P], bf16, tag="xchunk")
                            nc.sync.dma_start(
                                out=t_[:, :ntc * P],
                                in_=xT[kh, :, ch * P:(ch + ntc) * P])
                            ck.append(t_)
                        for j in range(ntc):
                            i = ch + j
                            ps = psAp.tile([P, W1C], f32, tag="psA")
                            nc.tensor.matmul(ps[:], ck[0][:, j * P:(j + 1) * P],
                                             w1t[0][:], start=True, stop=False)
                            nc.tensor.matmul(ps[:], ck[1][:, j * P:(j + 1) * P],
                                             w1t[1][:], start=False, stop=True)
                            row = rowp.tile([P, W1C], bf16, tag="row")
                            if i % 2 == 0:
                                nc.scalar.activation(row[:], ps[:], AF.Copy)
                            else:
                                nc.vector.tensor_copy(out=row[:], in_=ps[:])
                            nc.sync.dma_start(
                                out=hext[i * P:(i + 1) * P, 0:W1C], in_=row[:])

                sweep(xTA, hextA, RA)
                if RB:
                    sweep(xTB, hextB, RB)

                # ---------------- GAT conv layers ----------------
                def layer(lidx, tabA, tabB, nrowsA, nrowsB, srow, sc_win,
                          H, F, sc_src, sc_dst, sc_exp, epilogue, pools):
                    gp, gdp, wk, stp, psp, ep, pse = pools
                    for g, slots in enumerate(groups):
                        gbuf = [None, None]
                        for s in range(2):
                            col0, ntg = g_off[g][s]
                            if ntg == 0:
                                continue
                            tab = tabA if s == 0 else tabB
                            nrt = nrowsA if s == 0 else nrowsB
                            gt = gp.tile([P, ntg, srow], bf16, tag=f"g{s}")
                            inap = bass.AP(tensor=tab[:].tensor, offset=0,
                                           ap=[[srow, nrt], [1, srow]])
                            for c0 in range(0, ntg, 8):
                                cn = min(8, ntg - c0)
                                nc.gpsimd.dma_gather(
                                    gt[:, c0:c0 + cn, :], inap,
                                    gidx_t[:, col0 + c0 * 8:col0 + (c0 + cn) * 8],
                                    cn * P, cn * P, srow, elem_step=srow,
                                    queue_num=s)
                            gbuf[s] = gt
                        dcol0, dntg = d_off[g]
                        region = 0 if slots[0] < SLA else 1
                        dtab = tabA if region == 0 else tabB
                        dnr = nrowsA if region == 0 else nrowsB
                        gdt = gdp.tile([P, dntg, ROW2], bf16, tag="gd")
                        dinap = bass.AP(tensor=dtab[:].tensor, offset=sc_win,
                                        ap=[[srow, dnr], [1, ROW2]])
                        for c0 in range(0, dntg, 8):
                            cn = min(8, dntg - c0)
                            nc.gpsimd.dma_gather(
                                gdt[:, c0:c0 + cn, :], dinap,
                                dlidx_t[:, dcol0 + c0 * 8:dcol0 + (c0 + cn) * 8],
                                cn * P, cn * P, ROW2, elem_step=srow,
                                queue_num=2 + (c0 // 8) % 2)

                        goff = [0, 0]
                        doff = 0
                        for k in slots:
                            ntb = int(NT[k, 0] + NT[k, 1])
                            if ntb == 0:
                                continue
                            t0 = int(tile_of[k, 0])
                            al = wk.tile([P, NTMAX * H], f32, tag="al")
                            toff = 0
                            for s in range(2):
                                nts = int(NT[k, s])
                                if nts == 0:
                                    continue
                                src_ap = sub_ap(gbuf[s], goff[s] * srow + sc_src,
                                                [[srow, nts], [1, H]])
                                dst_ap = sub_ap(gdt, (doff + toff) * ROW2 + sc_dst,
                                                [[ROW2, nts], [1, H]])
                                out_ap = sub_ap(al, toff * H,
                                                [[H, nts], [1, H]])
                                nc.vector.tensor_tensor(
                                    out=out_ap, in0=src_ap, in1=dst_ap,
                                    op=OP.add)
                                toff += nts
                            nc.scalar.activation(al[:, :ntb * H],
                                                 al[:, :ntb * H],
                                                 AF.Prelu, alpha=NEG)
                            # exp -> score cols of the gathered rows
                            toff = 0
                            for s in range(2):
                                nts = int(NT[k, s])
                                if nts == 0:
                                    continue
                                eout = sub_ap(gbuf[s], goff[s] * srow + sc_exp,
                                              [[srow, nts], [1, H]])
                                ein = sub_ap(al, toff * H, [[H, nts], [1, H]])
                                nc.scalar.activation(eout, ein, AF.Exp)
                                toff += nts
                            # one-hot tiles for the whole block (batched)
                            st = stp.tile([P, NTMAX * P], bf16, tag="st")
                            nc.vector.tensor_tensor(
                                out=st[:, :ntb * P],
                                in0=iota_big[:, :ntb * P],
                                in1=sub_ap(dfp_t, t0, [[1, ntb], [0, P]]),
                                op=OP.is_equal)
                            # feats *= w (batched per split)
                            for s in range(2):
                                nts = int(NT[k, s])
                                if nts == 0:
                                    continue
                                gv = sub_ap(gbuf[s], goff[s] * srow,
                                            [[srow, nts], [F // H, H],
                                             [1, F // H]])
                                win = sub_ap(gbuf[s], goff[s] * srow + sc_exp,
                                             [[srow, nts], [1, H],
                                              [0, F // H]])
                                nc.vector.tensor_tensor(out=gv, in0=gv,
                                                        in1=win, op=OP.mult)
                            ps = psp.tile([P, F + H], f32, tag="ps")
                            ti = 0
                            for s in range(2):
                                nts = int(NT[k, s])
                                for j in range(nts):
                                    rhs = sub_ap(gbuf[s],
                                                 (goff[s] + j) * srow,
                                                 [[1, F + H]])
                                    nc.tensor.matmul(
                                        ps[:], st[:, ti * P:(ti + 1) * P],
                                        rhs, start=(ti == 0),
                                        stop=(ti == ntb - 1))
                                    ti += 1
                            epilogue(k, ps, ep, pse)
                            goff[0] += int(NT[k, 0])
                            goff[1] += int(NT[k, 1])
                            doff += ntb
                        if lidx == 1 and slots[-1] == SLA - 1:
                            nc.gpsimd.collective_compute(
                                "AllGather", mybir.AluOpType.bypass,
                                replica_groups=[list(range(NCORES))],
                                ins=[h2locA[:]], outs=[h2A[:]])

                def epi1(k, ps, ep, pse):
                    rden = ep.tile([P, H1], f32, tag="rden")
                    nc.vector.reciprocal(rden[:], ps[:, HID:HID + H1])
                    o = ep.tile([P, HID], bf16, tag="o")
                    o2d = bass.AP(tensor=o[:].tensor, offset=o[:].offset,
                                  ap=[o[:].ap[0], [C1, H1], [1, C1]])
                    num2 = bass.AP(tensor=ps[:].tensor, offset=ps[:].offset,
                                   ap=[ps[:].ap[0], [C1, H1], [1, C1]])
                    rb = sub_ap(rden, 0, [[1, H1], [0, C1]])
                    nc.vector.tensor_tensor(out=o2d, in0=num2, in1=rb,
                                            op=OP.mult)
                    if b1_t is not None:
                        nc.vector.tensor_tensor(out=o[:], in0=o[:],
                                                in1=b1_t[:], op=OP.add)
                    e = ep.tile([P, HID], bf16, tag="e")
                    nc.scalar.activation(e[:], o[:], AF.Exp)
                    nc.vector.tensor_scalar(out=o[:], in0=o[:], scalar1=0.0,
                                            scalar2=None, op0=OP.max)
                    nc.vector.tensor_scalar(out=e[:], in0=e[:], scalar1=1.0,
                                            scalar2=None, op0=OP.min)
                    nc.vector.tensor_tensor(out=o[:], in0=o[:], in1=e[:],
                                            op=OP.add)
                    h2ps = pse.tile([P, W2C], f32, tag="h2ps")
                    for half in range(2):
                        pt = pse.tile([P, P], bf16, tag="pt")
                        nc.tensor.transpose(pt[:],
                                            o[:, half * P:(half + 1) * P],
                                            ident[:])
                        et = ep.tile([P, P], bf16, tag="et")
                        nc.vector.tensor_copy(out=et[:], in_=pt[:])
                        nc.tensor.matmul(h2ps[:], et[:], w2t[half][:],
                                         start=(half == 0), stop=(half == 1))
                    h2r = ep.tile([P, ROW2], bf16, tag="h2r")
                    nc.vector.tensor_tensor(out=h2r[:, 0:OUT],
                                            in0=h2ps[:, 0:OUT],
                                            in1=ncs_t[:, 0:OUT], op=OP.add)
                    nc.vector.tensor_tensor(
                        out=h2r[:, OUT + 1:OUT + 3],
                        in0=h2ps[:, OUT:OUT + 2],
                        in1=ncs_t[:, OUT:OUT + 2], op=OP.add)
                    if k < SLA:
                        nc.sync.dma_start(
                            out=h2locA[k * P:(k + 1) * P, 0:OUT + 3],
                            in_=h2r[:, 0:OUT + 3])
                    else:
                        kk = k - SLA
                        nc.sync.dma_start(
                            out=h2locB[kk * P:(kk + 1) * P, 0:OUT + 3],
                            in_=h2r[:, 0:OUT + 3])

                def epi2(k, ps, ep, pse):
                    rden = ep.tile([P, 1], f32, tag="rden2")
                    nc.vector.reciprocal(rden[:], ps[:, OUT:OUT + 1])
                    o = ep.tile([P, OUT], f32, tag="o2")
                    nc.vector.tensor_scalar(out=o[:], in0=ps[:, 0:OUT],
                                            scalar1=rden[:], scalar2=None,
                                            op0=OP.mult)
                    if b2_t is not None:
                        nc.vector.tensor_tensor(out=o[:], in0=o[:],
                                                in1=b2_t[:], op=OP.add)
                    nc.sync.dma_start(out=out2[k * P:(k + 1) * P, :],
                                      in_=o[:])

                with (
                    tc.tile_pool(name="g1", bufs=2) as gp1,
                    tc.tile_pool(name="gd1", bufs=2) as gdp1,
                    tc.tile_pool(name="wk1", bufs=3) as wk1,
                    tc.tile_pool(name="st1", bufs=2) as stp1,
                    tc.tile_pool(name="ps1", bufs=2, space="PSUM") as psp1,
                    tc.tile_pool(name="ep1", bufs=3) as ep1,
                    tc.tile_pool(name="pse1", bufs=2, space="PSUM") as pse1,
                ):
                    layer(1, hextA, hextB, RA, RBp, SROW1, HID, H1, HID,
                          HID, H1, HID, epi1,
                          (gp1, gdp1, wk1, stp1, psp1, ep1, pse1))

            if RB:
                nc.gpsimd.collective_compute(
                    "AllGather", mybir.AluOpType.bypass,
                    replica_groups=[list(range(NCORES))],
                    ins=[h2locB[:]], outs=[h2B[:]])

            with (
                tc.tile_pool(name="g2", bufs=2) as gp2,
                tc.tile_pool(name="gd2", bufs=2) as gdp2,
                tc.tile_pool(name="wk2", bufs=3) as wk2,
                tc.tile_pool(name="st2", bufs=2) as stp2,
                tc.tile_pool(name="ps2", bufs=2, space="PSUM") as psp2,
                tc.tile_pool(name="ep2", bufs=3) as ep2,
                tc.tile_pool(name="pse2", bufs=2, space="PSUM") as pse2,
            ):
                layer(2, h2A, h2B, RA, RBp, ROW2, 0, 1, OUT, OUT + 1,
                      OUT + 2, OUT, epi2,
                      (gp2, gdp2, wk2, stp2, psp2, ep2, pse2))

    nc.finalize()
    return nc


def _host_prep_weights(W1, att1, W2, att2):
    HID = W1.shape[1]
    H1 = att1.shape[1]
    C1 = HID // H1
    OUT = W2.shape[1]
    A_src = np.zeros((HID, H1), np.float32)
    A_dst = np.zeros((HID, H1), np.float32)
    for h in range(H1):
        A_src[h * C1:(h + 1) * C1, h] = att1[0, h, C1:]
        A_dst[h * C1:(h + 1) * C1, h] = att1[0, h, :C1]
    W1ext = np.concatenate([W1, W1 @ A_src, W1 @ A_dst], axis=1)
    a2 = att2[0, 0]
    W2ext = np.concatenate([W2, (W2 @ a2[OUT:])[:, None],
                            (W2 @ a2[:OUT])[:, None]], axis=1)
    return W1ext, W2ext


def kernel(x, edge_index, W1, att1, b1, W2, att2, b2):
    from concourse import mybir
    from concourse.bass_utils import run_bass_kernel_spmd
    ml_bf16 = mybir.dt.np(mybir.dt.bfloat16)

    x = np.asarray(x, np.float32)
    edge_index = np.asarray(edge_index)
    W1 = np.asarray(W1, np.float32)
    att1 = np.asarray(att1, np.float32)
    b1 = np.asarray(b1, np.float32)
    W2 = np.asarray(W2, np.float32)
    att2 = np.asarray(att2, np.float32)
    b2 = np.asarray(b2, np.float32)

    N, IN = x.shape
    HID = W1.shape[1]
    H1 = att1.shape[1]
    C1 = HID // H1
    OUT = W2.shape[1]

    plan, per_core = _prep(edge_index, N)
    NB, SLA, SLB = plan["NB"], plan["SLA"], plan["SLB"]
    RA, RB = plan["RA"], plan["RB"]
    dims = dict(IN=IN, HID=HID, H1=H1, C1=C1, OUT=OUT)
    has_b1 = bool(np.any(b1 != 0))
    has_b2 = bool(np.any(b2 != 0))

    key = (N, IN, HID, H1, OUT, plan["g_cols"], plan["d_cols"], plan["NTOT"],
           has_b1, has_b2, tuple(int(v) for v in plan["NT"].ravel()))
    if key not in _CACHE:
        _CACHE[key] = _build(plan, dims, has_b1, has_b2)
    nc = _CACHE[key]

    W1ext, W2ext = _host_prep_weights(W1, att1, W2, att2)
    ncs = np.tile(-W2ext.sum(axis=0, keepdims=True), (P, 1)).astype(np.float32)

    def xt_table(ids, mask, rows):
        xt = x[np.clip(ids, 0, N - 1)] * mask[:, None]
        return np.ascontiguousarray(xt.T).reshape(2, P, rows).astype(ml_bf16)

    xTA = xt_table(plan["idsA"], plan["maskA"], RA)
    xTB = xt_table(plan["idsB"], plan["maskB"], max(RB, 1)) if RB else None

    def ktiles(w):
        return np.ascontiguousarray(w.reshape(2, P, -1)).astype(ml_bf16)

    in_maps = []
    for c in range(NCORES):
        m = dict(
            xTA=xTA,
            w1e=ktiles(W1ext),
            w2e=ktiles(W2ext),
            negcs=ncs,
            g_idx=per_core[c]["g_idx"],
            dl_idx=per_core[c]["dl_idx"],
            d_fp=per_core[c]["d_fp"].astype(ml_bf16),
        )
        if RB:
            m["xTB"] = xTB
        if has_b1:
            m["b1r"] = np.tile(b1[None, :], (P, 1)).astype(ml_bf16)
        if has_b2:
            m["b2r"] = np.tile(b2[None, :], (P, 1)).astype(np.float32)
        in_maps.append(m)

    trace = bool(os.environ.get("GAT_TRACE"))
    global LAST_NC, LAST_IN_MAPS
    LAST_NC, LAST_IN_MAPS = nc, in_maps
    res = run_bass_kernel_spmd(nc, in_maps, list(range(NCORES)), trace=trace)
    global LAST_RESULTS
    LAST_RESULTS = res

    out = np.zeros((N, OUT), np.float32)
    block_of = plan["block_of"]
    for c in range(NCORES):
        o = res.results[c]["out2"]
        for k in range(NB):
            B = int(block_of[c, k])
            if B < 0:
                continue
            lo = B * P
            hi = min(lo + P, N)
            out[lo:hi] = o[k * P:k * P + (hi - lo)]
    return np.ascontiguousarray(out)
